# revision 5
# baseline (speedup 1.0000x reference)
"""SIR ODE batch integrator on 8 Trainium2 NeuronCores (Bass/Tile) — v2.

Strategy (vs v1): minimize DVE op count and per-op cost.
  - 2-state fp32 [S | C] (C = S+I), as v1.
  - Increments in fp16: X = [S*I | I] (custom DVE op), K = C_h (.) X
    (fp16 tensor_tensor -> 2x_1p DVE mode, ~127ns vs 194ns at 1x).
  - AB2 tail in V-form with constant h = 100/199:
        Y_{n+1} = V_n + K''_n          (DVE, mixed f32/f16 add)
        V_{n+1} = V_n + (2/3) K''_n    (GPSIMD stt — off the DVE)
    where K'' = 1.5h*K(Y_n), V_n = Y_n - 0.5h*K(Y_{n-1}).  DVE critical
    path is 3 ops/interval (~515ns) instead of v1's 4 (~776ns).
  - Short bootstrap: 2 RK4 + 4 SSPRK3 + 2 midpoint intervals (fp16
    increments), then AB2 for the remaining 191.  Bit-exact NumPy model
    of this op sequence measures rel fro-norm 9.3e-4, absmax 1.6e-2 vs
    the fp32 reference (gate: rel < 2e-2).
  - Output: Y written directly into 4-interval-wide staging tiles; one
    DMA per 4 intervals (50 total) keeps the SP sequencer (~565ns per
    DMA dispatch) off the critical path.
"""

import numpy as np

try:
    import concourse.bass as bass
except ImportError:  # pragma: no cover - container default location
    import sys

    sys.path.insert(0, "/opt/trn_rl_repo")
    import concourse.bass as bass

import concourse.bacc as bacc
import concourse.mybir as mybir
from concourse.tile import TileContext
from concourse.bass_utils import run_bass_kernel_spmd

F32 = mybir.dt.float32
F16 = mybir.dt.float16
AL = mybir.AluOpType


def _register_ti_op():
    """Custom DVE op computing X = [t | I] from Y = [S | C] in ONE wide
    instruction: in0 = Y, in1 = column-block-swapped Y (= [C | S]).
    With r = Src1 - Src0:
      k <  F (Src0=S, Src1=C): out = r*Src0 = (C-S)*S = S*I   (t half)
      k >= F (Src0=C, Src1=S): out = 0-r    = C-S = I         (I half)
    """
    import numpy as _np
    from concourse import dve_ops as _dve_ops
    from concourse.dve_spec import Spec, Src0, Src1, C0, Zero, Idx, select, lower
    from concourse.dve_uop import DveOpSpec

    name = "SIR_TI_FUSED"
    for op in _dve_ops.OPS:
        if op.name == name:
            return op
    r = Src1 - Src0

    def _ref(in0, in1, s0):
        idx = _np.arange(in0.shape[-1], dtype=_np.float32)
        rr = in1 - in0
        return _np.where(idx < s0, rr * in0, -rr)

    spec = Spec(body=select(Idx < C0, r * Src0, Zero - r), reference=_ref)
    row = _dve_ops._CUSTOM_DVE_ROW_BASE + len(_dve_ops.OPS)
    assert row < 0x20
    shas = {
        ver: DveOpSpec(
            name=name, opcode=row, uops=lower(spec, ver=ver), rd1_en=True
        ).sha(ver)
        for ver in ("v3", "v4")
    }
    op = _dve_ops.DveOp(name, spec, subdim=False, uops_sha=shas)
    _dve_ops.OPS.append(op)
    _dve_ops.CUSTOM_DVE_SPECS[name] = spec
    _dve_ops._SUB_OPCODE_FOR_NAME[name] = row
    return op


_TI_OP = _register_ti_op()

N_CORES = 8
B = 65536
PER = B // N_CORES  # 8192 samples per core
P = 128
F = PER // P  # 64
W = 2 * F  # wide tile free size (S and C halves)
NUM_T = 200
NI = NUM_T - 1  # 199 intervals

N_RK4 = 2
N_SSP = 3
N_MID = 1
N_BOOT = N_RK4 + N_SSP + N_MID  # 8
BATCH = 4  # intervals per output DMA; first batch is 3 (3 + 49*4 = 199)


class _Builder:
    """Emits the fp16-increment integrator ops.  All state/const args are
    plain APs; fp16 work tiles come from `wpool`."""

    def __init__(self, nc, wpool, c_h):
        self.nc = nc
        self.w = wpool
        self.c_h = c_h  # fp16 const AP [-b*h | -g*h]

    def evalX(self, Ys, tag="X"):
        X = self.w.tile([P, W], F16, tag=tag, name="X")
        Yrev = Ys.rearrange("p (two f) -> p two f", two=2)[:, ::-1, :]
        self.nc.vector._custom_dve(_TI_OP, out=X[:], in0=Ys, in1=Yrev, s0=float(F))
        return X[:]

    def evalK(self, Ys, stage=1):
        X = self.evalX(Ys, tag=f"X{stage}")
        K = self.w.tile([P, W], F16, tag=f"K{stage}", name="K")
        self.nc.vector.tensor_tensor(K[:], self.c_h, X, AL.mult)
        return X, K[:]

    def ts(self, src, scale, tag):
        T = self.w.tile([P, W], F16, tag=tag)
        self.nc.vector.tensor_scalar_mul(T[:], src, float(scale))
        return T[:]

    def add16(self, a, b, tag):
        T = self.w.tile([P, W], F16, tag=tag)
        self.nc.vector.tensor_tensor(T[:], a, b, AL.add)
        return T[:]

    def ymix(self, Ys, inc, out_ap):
        """out = Ys (fp32) + inc (fp16)."""
        self.nc.vector.tensor_tensor(out_ap, Ys, inc, AL.add)

    def ytmp(self, Ys, inc, tag):
        T = self.w.tile([P, W], F32, tag=tag)
        self.ymix(Ys, inc, T[:])
        return T[:]


def _rk4(bld, Ys, out_ap):
    X1, K1 = bld.evalK(Ys, 1)
    Y2 = bld.ytmp(Ys, bld.ts(K1, 0.5, "T1"), "Y2")
    X2, K2 = bld.evalK(Y2, 2)
    Y3 = bld.ytmp(Ys, bld.ts(K2, 0.5, "T2"), "Y3")
    X3, K3 = bld.evalK(Y3, 3)
    Y4 = bld.ytmp(Ys, K3, "Y4")
    X4, K4 = bld.evalK(Y4, 4)
    A1 = bld.add16(K1, K4, "A1")
    A2 = bld.add16(K2, K3, "A2")
    T2 = bld.ts(A2, 2.0, "T3")
    A3 = bld.add16(A1, T2, "A3")
    T3 = bld.ts(A3, 1.0 / 6.0, "T4")
    bld.ymix(Ys, T3, out_ap)


def _ssprk3(bld, Ys, out_ap):
    X1, K1 = bld.evalK(Ys, 1)
    Y2 = bld.ytmp(Ys, K1, "Y2")
    X2, K2 = bld.evalK(Y2, 2)
    A1 = bld.add16(K1, K2, "A1")
    Y3 = bld.ytmp(Ys, bld.ts(A1, 0.25, "T1"), "Y3")
    X3, K3 = bld.evalK(Y3, 3)
    A2 = bld.add16(bld.ts(K3, 4.0, "T2"), A1, "A2")
    T3 = bld.ts(A2, 1.0 / 6.0, "T3")
    bld.ymix(Ys, T3, out_ap)


def _mid(bld, Ys, out_ap):
    """Midpoint RK2.  Returns X1 = X(Ys) (AB2 history)."""
    X1, K1 = bld.evalK(Ys, 1)
    Y2 = bld.ytmp(Ys, bld.ts(K1, 0.5, "T1"), "Y2")
    X2, K2 = bld.evalK(Y2, 2)
    bld.ymix(Ys, K2, out_ap)
    return X1


def build_nc(reps=1, use_gpsimd=False):
    # use_gpsimd=False: the Pool engine shares an SBUF port with the DVE, so
    # offloading the W update there measures ~30us SLOWER end-to-end than
    # keeping all four ops on the DVE (253us vs 222us).
    nc = bacc.Bacc(None)
    pin32 = nc.declare_dram_parameter("pin32", [P, W], F32, isOutput=False)
    pin16 = nc.declare_dram_parameter("pin16", [P, 3 * W], F16, isOutput=False)
    # Output stays in staging layout (one [P, n*W] row per DMA batch) so
    # every DMA is a plain contiguous copy; host unpacks.  Intervals 0-2 go
    # to out_a, 3..198 to out_b in 49 batches of 4.
    out_a = nc.declare_dram_parameter("out_a", [P, 3 * W], F32, isOutput=True)
    out_b = nc.declare_dram_parameter("out_b", [49, P, BATCH * W], F32, isOutput=True)

    with TileContext(nc) as tc:
        with (
            tc.tile_pool(name="const", bufs=1) as cpool,
            tc.tile_pool(name="stag", bufs=3) as spool,
            tc.tile_pool(name="vpool", bufs=3) as vpool,
            tc.tile_pool(name="work", bufs=2) as wpool,
        ):

            def body(_=None):
                p32 = cpool.tile([P, W], F32, tag="p32")
                nc.sync.dma_start(out=p32[:], in_=pin32[:])
                p16 = cpool.tile([P, 3 * W], F16, tag="p16")
                nc.sync.dma_start(out=p16[:], in_=pin16[:])
                c_h = p16[:, 0:W]  # [-b*h | -g*h]
                c_a = p16[:, W : 2 * W]  # 1.5x that
                c_m = p16[:, 2 * W : 3 * W]  # -0.5x that

                bld = _Builder(nc, wpool, c_h)

                # staging state: (tile, first interval idx, n slots, cur slot)
                st = {"t": None}

                def slot_ap(k):
                    if st["t"] is None:
                        n = 3 if k == 0 else BATCH
                        tg = "stag0" if k == 0 else "stag"
                        st["t"] = spool.tile([P, n * W], F32, tag=tg, name="stag")
                        st["first"] = k
                        st["n"] = n
                        st["s"] = 0
                    s = st["s"]
                    return st["t"][:, s * W : (s + 1) * W]

                def advance(k):
                    st["s"] += 1
                    if st["s"] == st["n"]:
                        first = st["first"]
                        if first == 0:
                            nc.sync.dma_start(out=out_a[:], in_=st["t"][:])
                        else:
                            nc.sync.dma_start(
                                out=out_b[(first - 3) // BATCH], in_=st["t"][:]
                            )
                        st["t"] = None

                Y = p32[:]
                k = 0
                Xhist = None
                for i in range(N_RK4):
                    ap = slot_ap(k)
                    _rk4(bld, Y, ap)
                    Y = ap
                    advance(k)
                    k += 1
                for i in range(N_SSP):
                    ap = slot_ap(k)
                    _ssprk3(bld, Y, ap)
                    Y = ap
                    advance(k)
                    k += 1
                for i in range(N_MID):
                    ap = slot_ap(k)
                    Xhist = _mid(bld, Y, ap)
                    Y = ap
                    advance(k)
                    k += 1
                # --- W bootstrap: W = 1.5*(Y + c_m (.) Xhist)  (W = 1.5*V,
                # V_n = Y_n - 0.5h*K_{n-1}; scaling makes the per-interval
                # W update a pure add, which the Pool engine supports) ---
                T = wpool.tile([P, W], F16, tag="Tv")
                nc.vector.tensor_tensor(T[:], c_m, Xhist, AL.mult)
                V0 = vpool.tile([P, W], F32, tag="V0")
                nc.vector.tensor_tensor(V0[:], Y, T[:], AL.add)
                Wst = vpool.tile([P, W], F32, tag="W", name="Wst")
                nc.vector.tensor_scalar_mul(Wst[:], V0[:], 1.5)
                # --- AB2 tail:
                #   Y_{n+1} = (2/3) W_n + K''_n   (DVE stt)
                #   W_{n+1} = W_n + K''_n         (GPSIMD add, off DVE) ---
                while k < NI:
                    X = bld.evalX(Y)
                    Kpp = wpool.tile([P, W], F16, tag="Kpp", bufs=4, name="Kpp")
                    nc.vector.tensor_tensor(Kpp[:], c_a, X, AL.mult)
                    ap = slot_ap(k)
                    nc.vector.scalar_tensor_tensor(
                        ap, Wst[:], 2.0 / 3.0, Kpp[:], AL.mult, AL.add
                    )
                    Wn = vpool.tile([P, W], F32, tag="W", name="Wn")
                    eng = nc.gpsimd if use_gpsimd else nc.vector
                    eng.tensor_tensor(Wn[:], Wst[:], Kpp[:], AL.add)
                    Wst = Wn
                    Y = ap
                    advance(k)
                    k += 1

            if reps == 1:
                body()
            else:
                with tc.For_i(0, reps, 1):
                    body()
    nc.finalize()
    return nc


def pack_inputs(params: np.ndarray) -> list[dict]:
    params = np.asarray(params, dtype=np.float32)
    hd = np.float64(100.0) / 199.0
    in_maps = []
    for c in range(N_CORES):
        sl = params[c * PER : (c + 1) * PER]
        b64 = sl[:, 0].astype(np.float64)
        g64 = sl[:, 1].astype(np.float64)
        p32 = np.empty((P, W), dtype=np.float32)
        p32[:, 0:F] = sl[:, 2].reshape(P, F)  # S0
        p32[:, F:W] = (sl[:, 2] + sl[:, 3]).reshape(P, F)  # C0
        p16 = np.empty((P, 3 * W), dtype=np.float16)
        for j, scale in enumerate((hd, 1.5 * hd, -0.5 * hd)):
            p16[:, j * W + 0 : j * W + F] = (-b64 * scale).reshape(P, F)
            p16[:, j * W + F : j * W + W] = (-g64 * scale).reshape(P, F)
        in_maps.append({"pin32": p32, "pin16": p16})
    return in_maps


_NC_CACHE = {}


def kernel(params: np.ndarray) -> np.ndarray:
    params = np.asarray(params, dtype=np.float32)
    assert params.shape == (B, 4)

    if "nc" not in _NC_CACHE:
        _NC_CACHE["nc"] = build_nc()
    nc = _NC_CACHE["nc"]

    in_maps = pack_inputs(params)
    res = run_bass_kernel_spmd(nc, in_maps, list(range(N_CORES)))

    out_full = np.empty((B, NUM_T, 3), dtype=np.float32)
    one = np.float32(1.0)
    S0 = params[:, 2]
    I0 = params[:, 3]
    out_full[:, 0, 0] = S0
    out_full[:, 0, 1] = I0
    out_full[:, 0, 2] = (one - S0) - I0
    for c in range(N_CORES):
        oa = res.results[c]["out_a"]  # [P, 3W]
        ob = res.results[c]["out_b"]  # [49, P, BATCH*W]
        o = np.empty((NI, P, W), dtype=np.float32)
        o[0:3] = oa.reshape(P, 3, W).transpose(1, 0, 2)
        o[3:] = (
            ob.reshape(49, P, BATCH, W).transpose(0, 2, 1, 3).reshape(196, P, W)
        )
        S = o[:, :, :F].reshape(NI, PER).T  # [PER, NI]
        C = o[:, :, F:].reshape(NI, PER).T
        blk = out_full[c * PER : (c + 1) * PER]
        blk[:, 1:, 0] = S
        blk[:, 1:, 1] = C - S
        blk[:, 1:, 2] = one - C
    return out_full


if __name__ == "__main__":
    rng = np.random.RandomState(0)
    p = rng.uniform(0, 1, (B, 4)).astype(np.float32)
    r = kernel(p)
    print(r.shape, r.dtype, r[0, :3], flush=True)
